# revision 19
# baseline (speedup 1.0000x reference)
"""CapsuleLayer (dynamic routing, 3 iterations) on 8 Trainium2 NeuronCores.

Zero-collective design. A collective-based kernel spends ~60us waiting for
the ncfw/TOPSP firmware to boot plus ~10us per collective; with ~25us of
real math that dominates. This kernel eliminates every collective:

  - The routing statistics (b_ij += mean over 256 batch samples of u_hat.v)
    tolerate large per-element noise (it averages out ~16x in the batch
    mean), so each core REPLICATES the full-batch routing (iterations 1-2)
    in fp8 instead of sharding it (measured ~3e-3 final rel err vs the 2e-2
    gate, identical to bf16 routing).
  - Iteration 3 (output-determining) runs in bf16 with each core producing
    only its 32-row batch shard of v_3; the host concatenates. No
    ReduceScatter, no AllGather, no warm-up, no ncfw boot.

Layout: rows j = (i,k), 9216 rows = 72 chunks of 128. All DRAM inputs are
host-packed partition-major so every DMA is contiguous (~380 GB/s measured
vs ~96 for the gather layout). One DGE issuer (sync) serializes the waves
in priority order: (wl8T,xt8 for s1) -> (xik8,wlb for Q1) -> (wlbT, xts).

Engine facts this schedule is built on (all measured on this hardware):
fp8 DoubleRow matmul 69ns warm (256-deep contraction); DVE 0.54 ns/elem
only when every operand is 2-byte, packed, SBUF (broadcast-over-innermost
or PSUM reads drop it to 1.07-1.37; windowed reduce is always 1.07; a
concurrently streaming PE degrades DVE SBUF access ~2.5x); GpSimd 1.95;
Scalar copy 1.2-1.37.
  s/Q matmuls   fp8 DoubleRow; rhs (wl/mc) kept in TRANSPOSED [p,(n,o),c]
                layout so the mc build multiplies are packed along c.
  mc = c o wl   DVE packed multiplies (cT broadcast over o sits OUTSIDE the
                innermost c dim), ~1.2us per 12-chunk slab.
  Q egress      Scalar copies 3 of 4 PSUM banks to bf16 (DVE direct-
                multiplies the 4th); DVE packed-multiplies p = wlb o Q.
  pr = sum_o p  packed bf16 add-tree 16->1; level 1 of the first half on
                GpSimd (hidden under remaining Q groups), rest on DVE.
  uv = F.T @ pr PE; F = kron(I16, ones8x8)/(B*SV) = 2^-12 exact in bf16
                (sums k inside i-groups, replicates back, folds all scales)
  s matmuls     batch-half-split: squash(half 0) overlaps PE on half 1.
Scales: wl8 = 16*0.03*W keeps fp8 normals; v8 = 16*v; x unscaled; the
final squash folds SV into the last Newton step's constants.
"""
import sys

if "/opt/trn_rl_repo" not in sys.path:
    sys.path.insert(0, "/opt/trn_rl_repo")

import numpy as np

N_CORES = 8
B, IN_SIZE, I_TOT = 256, 8, 1152
N_NODE, O_SZ = 10, 16
NO = N_NODE * O_SZ          # 160
J = I_TOT * IN_SIZE         # 9216 rows (i,k)
NCH = J // 128              # 72 chunks
NG = NCH // 2               # 36 DoubleRow chunk-pairs
NH = NCH // 2               # 36 chunks per b_update half
B_SH = B // N_CORES         # 32 batch rows per core
SW = 16.0                   # wl fp8 scale
SV = 16.0                   # v fp8 scale
RSQRT_MAGIC = 0x5F3759DF

_CACHE = {}


def _build_program():
    import concourse.bacc as bacc
    import concourse.tile as tile
    import concourse.mybir as mybir

    f32 = mybir.dt.float32
    bf16 = mybir.dt.bfloat16
    f8 = mybir.dt.float8e4
    i32 = mybir.dt.int32
    AF = mybir.ActivationFunctionType
    ALU = mybir.AluOpType
    AX = mybir.AxisListType
    PM = mybir.MatmulPerfMode.DoubleRow

    nc = bacc.Bacc("TRN2", target_bir_lowering=False, debug=False,
                   enable_asserts=True, num_devices=N_CORES)

    xt8_d = nc.dram_tensor("xt8", [128, NCH * B], f8,
                           kind="ExternalInput").ap()
    xik8_d = nc.dram_tensor("xik8", [128, 2 * J], f8,
                            kind="ExternalInput").ap()
    wl8_d = nc.dram_tensor("wl8", [128, NCH * NO], f8,
                           kind="ExternalInput").ap()
    wlb_d = nc.dram_tensor("wlb", [128, NCH * NO], bf16,
                           kind="ExternalInput").ap()
    xts_d = nc.dram_tensor("xts", [128, NCH * B_SH], bf16,
                           kind="ExternalInput").ap()
    f_d = nc.dram_tensor("fmat", [128, 128], bf16, kind="ExternalInput").ap()
    y_d = nc.dram_tensor("y", [B_SH, NO], f32, kind="ExternalOutput").ap()

    with tile.TileContext(nc) as tc:
        with tc.tile_pool(name="persist", bufs=1) as pp, \
             tc.tile_pool(name="work", bufs=1) as wp, \
             tc.tile_pool(name="half", bufs=1) as hp, \
             tc.tile_pool(name="ps_s", bufs=1, space="PSUM") as ps_s, \
             tc.tile_pool(name="ps_q", bufs=3, space="PSUM") as ps_q, \
             tc.tile_pool(name="ps_f", bufs=1, space="PSUM") as ps_f:

            xt8_sb = pp.tile([128, NCH, B], f8, name="xt8_sb", tag="xt8_sb")
            xik8_sb = pp.tile([128, 2, J], f8, name="xik8_sb", tag="xik8_sb")
            wl8_sb = pp.tile([128, NCH, NO], f8, name="wl8_sb",
                             tag="wl8_sb")
            wlb_sb = pp.tile([128, NCH, NO], bf16, name="wlb_sb",
                             tag="wlb_sb")
            xts_sb = pp.tile([128, NCH, B_SH], bf16, name="xts_sb",
                             tag="xts_sb")
            f_sb = pp.tile([128, 128], bf16, name="f_sb", tag="f_sb")
            b_sb = pp.tile([128, NCH, N_NODE], f32, name="b_sb", tag="b_sb")

            # ---------------- input loads ----------------
            xt8f = xt8_sb[:].rearrange("p c b -> p (c b)")
            wl8f = wl8_sb[:].rearrange("p c f -> p (c f)")
            wlbf = wlb_sb[:].rearrange("p c f -> p (c f)")
            xikf = xik8_sb[:].rearrange("p t j -> p (t j)")
            xtsf = xts_sb[:].rearrange("p c b -> p (c b)")
            nc.gpsimd.dma_start(f_sb[:], f_d[:])
            SL = NCH // 4
            for si in range(4):  # wave 1: s1 inputs, chunk-interleaved
                cs = slice(si * SL * NO, (si + 1) * SL * NO)
                nc.sync.dma_start(wl8f[:, cs], wl8_d[:, cs])
                cs = slice(si * SL * B, (si + 1) * SL * B)
                nc.sync.dma_start(xt8f[:, cs], xt8_d[:, cs])
            for qi in range(4):  # wave 2: Q1 inputs
                js = slice(qi * J // 2, (qi + 1) * J // 2)
                nc.sync.dma_start(xikf[:, js], xik8_d[:, js])
                ws = slice(qi * NCH // 4 * NO, (qi + 1) * NCH // 4 * NO)
                nc.sync.dma_start(wlbf[:, ws], wlb_d[:, ws])
            nc.sync.dma_start(xtsf[:], xts_d[:])  # wave 3: iter-3 input

            # prewarm the Exp ACT table during the DMA wait
            warm = wp.tile([128, 1], f32, name="warm", tag="warm")
            nc.vector.memset(warm[:], 0.0)
            nc.scalar.activation(warm[:], warm[:], AF.Exp)

            wl84 = wl8_sb[:].rearrange("p c (n o) -> p c n o", n=N_NODE)
            wlb4 = wlb_sb[:].rearrange("p c (n o) -> p c n o", n=N_NODE)

            # ---------------- helpers ----------------

            def squash_half(s_src, v_out, P, nch, tag, fac_scale,
                            newton_iters=1):
                """v_out = squash(s_src) over o; fac_scale folded into the
                last Newton step (exact: pow2)."""
                s4 = s_src.rearrange("p c (n o) -> p c n o", n=N_NODE)
                sq = wp.tile([P, nch, NO], f32, name="sq" + tag,
                             tag="sq" + tag)
                nc.vector.tensor_mul(sq[:], s_src, s_src)
                msq = wp.tile([P, nch, N_NODE], f32, name="msq" + tag,
                              tag="msq" + tag)
                nc.vector.reduce_sum(
                    msq[:], sq[:].rearrange("p c (n o) -> p c n o",
                                            n=N_NODE),
                    axis=AX.X)
                zi = wp.tile([P, nch, N_NODE], i32, name="zi" + tag,
                             tag="zi" + tag)
                nc.vector.tensor_scalar(
                    out=zi[:], in0=msq[:].bitcast(i32), scalar1=1, scalar2=-1,
                    op0=ALU.arith_shift_right, op1=ALU.bitwise_xor)
                nc.vector.tensor_scalar_add(zi[:], zi[:], RSQRT_MAGIC + 1)
                z = zi[:].bitcast(f32)
                t = wp.tile([P, nch, N_NODE], f32, name="nt" + tag,
                            tag="nt" + tag)
                w = wp.tile([P, nch, N_NODE], f32, name="nw" + tag,
                            tag="nw" + tag)
                for it in range(newton_iters):
                    last = it == newton_iters - 1
                    fs = fac_scale if last else 1.0
                    nc.vector.tensor_mul(t[:], z, z)
                    nc.vector.tensor_mul(t[:], t[:], msq[:])
                    nc.vector.tensor_scalar(
                        out=w[:], in0=t[:], scalar1=-0.5 * fs,
                        scalar2=1.5 * fs, op0=ALU.mult, op1=ALU.add)
                    nc.vector.tensor_mul(z, z, w[:])
                mag = wp.tile([P, nch, N_NODE], f32, name="mag" + tag,
                              tag="mag" + tag)
                nc.vector.tensor_mul(mag[:], msq[:], z)  # fs*sqrt(msq)
                den = wp.tile([P, nch, N_NODE], f32, name="den" + tag,
                              tag="den" + tag)
                nc.vector.tensor_scalar_add(den[:], msq[:], 1.0)
                rden = wp.tile([P, nch, N_NODE], f32, name="rden" + tag,
                               tag="rden" + tag)
                nc.vector.reciprocal(rden[:], den[:])
                fac = wp.tile([P, nch, N_NODE], f32, name="fac" + tag,
                              tag="fac" + tag)
                nc.vector.tensor_mul(fac[:], mag[:], rden[:])
                fb = fac[:].unsqueeze(3).broadcast_to(
                    (P, nch, N_NODE, O_SZ))
                nc.vector.tensor_mul(
                    v_out.rearrange("p c (n o) -> p c n o", n=N_NODE),
                    s4, fb)

            def s_iter(rhs_sb, scale, v8_sb):
                bank = [ps_s.tile([128, NO], f32, name=f"s_ps{bc}",
                                  tag=f"s_ps{bc}") for bc in range(2)]
                for g in range(NG):
                    for bc in range(2):
                        nc.tensor.matmul(
                            bank[bc][:],
                            xt8_sb[:, 2 * g:2 * g + 2,
                                   bc * 128:(bc + 1) * 128],
                            rhs_sb[:, 2 * g:2 * g + 2, :],
                            start=(g == 0), stop=(g == NG - 1),
                            perf_mode=PM)
                s_sb = wp.tile([128, 2, NO], f32, name="s_sb", tag="s_sb")
                for bc in range(2):
                    nc.scalar.mul(s_sb[:, bc, :], bank[bc][:], scale)
                squash_half(s_sb[:], v8_sb[:], 128, 2, "m", SV)

            def half_tree(ph4, prb, h):
                t8 = hp.tile([128, NH, N_NODE, 8], bf16, name="t8",
                             tag="t8" + str(h))
                nc.vector.tensor_add(t8[:], ph4[..., 0:8], ph4[..., 8:16])
                t4 = hp.tile([128, NH, N_NODE, 4], bf16, name="t4",
                             tag="t4" + str(h))
                nc.vector.tensor_add(t4[:], t8[:, :, :, 0:4],
                                     t8[:, :, :, 4:8])
                t2 = hp.tile([128, NH, N_NODE, 2], bf16, name="t2",
                             tag="t2" + str(h))
                nc.vector.tensor_add(t2[:], t4[:, :, :, 0:2],
                                     t4[:, :, :, 2:4])
                nc.vector.tensor_add(
                    prb[:, h * NH:(h + 1) * NH, :].unsqueeze(3),
                    t2[:, :, :, 0:1], t2[:, :, :, 1:2])

            def b_update(v8_sb, first, mc_half=None):
                prb = wp.tile([128, NCH, N_NODE], bf16, name="prb",
                              tag="prb")
                for h in range(2):
                    ph = hp.tile([128, NH, NO], bf16, name="ph",
                                 tag="ph" + str(h))
                    for r in range(3):
                        qrun = hp.tile([128, 9, NO], bf16, name="qrun",
                                       tag="qr" + str(r % 2))
                        for gi in range(4):
                            gq = h * 12 + r * 4 + gi
                            q_ps = ps_q.tile([128, 3 * NO], f32,
                                             name="q_ps", tag="q_ps")
                            for s_i in range(3):
                                mch = gq * 3 + s_i
                                nc.tensor.matmul(
                                    q_ps[:, s_i * NO:(s_i + 1) * NO],
                                    xik8_sb[:, :,
                                            mch * 128:(mch + 1) * 128],
                                    v8_sb[:],
                                    start=True, stop=True, perf_mode=PM)
                            q3 = q_ps[:].rearrange("p (c f) -> p c f", c=3)
                            lo = (r * 4 + gi) * 3
                            if gi == 3:
                                nc.vector.tensor_mul(
                                    ph[:, lo:lo + 3, :],
                                    wlb_sb[:,
                                           h * NH + lo:h * NH + lo + 3, :],
                                    q3)
                            else:
                                nc.scalar.copy(
                                    qrun[:, gi * 3:gi * 3 + 3, :], q3)
                        lo = r * 12
                        nc.vector.tensor_mul(
                            ph[:, lo:lo + 9, :],
                            wlb_sb[:, h * NH + lo:h * NH + lo + 9, :],
                            qrun[:])
                    ph4 = ph[:].rearrange("p c (n o) -> p c n o", n=N_NODE)
                    half_tree(ph4, prb, h)
                    uv_ps = ps_f.tile([128, NH * N_NODE], f32,
                                      name=f"uv_ps{h}", tag=f"uv_ps{h}")
                    nc.tensor.matmul(
                        uv_ps[:], f_sb[:],
                        prb[:, h * NH:(h + 1) * NH, :]
                        .rearrange("p c n -> p (c n)"),
                        start=True, stop=True)
                    uv3 = uv_ps[:].rearrange("p (c n) -> p c n", n=N_NODE)
                    hs = slice(h * NH, (h + 1) * NH)
                    if first:
                        nc.scalar.copy(b_sb[:, hs, :], uv3)
                        b_src = uv3
                    else:
                        nc.vector.tensor_add(b_sb[:, hs, :],
                                             b_sb[:, hs, :], uv3)
                        b_src = b_sb[:, hs, :]
                    softmax_half(h, b_src)
                    if mc_half is not None:
                        mc_half(h)
                return None

            e_sb = pp.tile([128, NCH, N_NODE], f32, name="e_sb",
                           tag="e_sb")
            se = pp.tile([128, NCH], f32, name="se", tag="se")
            rse = pp.tile([128, NCH], f32, name="rse", tag="rse")
            c_sb = pp.tile([128, NCH, N_NODE], bf16, name="c_sb",
                           tag="c_sb")

            def softmax_half(h, b_src):
                hs = slice(h * NH, (h + 1) * NH)
                nc.scalar.activation(e_sb[:, hs, :], b_src, AF.Exp)
                nc.vector.reduce_sum(se[:, hs], e_sb[:, hs, :], axis=AX.X)
                nc.vector.reciprocal_approx_fast(rse[:, hs], se[:, hs])
                nc.vector.tensor_mul(
                    c_sb[:, hs, :], e_sb[:, hs, :],
                    rse[:, hs].unsqueeze(2).broadcast_to(
                        (128, NH, N_NODE)))

            def mc_half_fn(mc, wl4_src):
                mc4 = mc[:].rearrange("p c (n o) -> p c n o", n=N_NODE)
                cb = c_sb[:].unsqueeze(3).broadcast_to(
                    (128, NCH, N_NODE, O_SZ))

                def go(h):
                    # h0: only the two GpSimd slabs (they hide under h1's
                    # update; a DVE slab here would block h1's DVE chain in
                    # the in-order queue). h1: DVE takes 2,3,4 after the
                    # critical path; GpSimd takes the last-consumed slab 5.
                    if h == 0:
                        slabs = [(0, nc.gpsimd), (1, nc.gpsimd)]
                    else:
                        slabs = [(2, nc.vector), (3, nc.vector),
                                 (4, nc.vector), (5, nc.gpsimd)]
                    for sl, eng in slabs:
                        cs = slice(sl * 12, (sl + 1) * 12)
                        eng.tensor_mul(mc4[:, cs], wl4_src[:, cs],
                                       cb[:, cs])
                return go

            # ---------------- iteration 1 (c uniform = 0.1) ----------------
            v8 = wp.tile([128, 2, NO], f8, name="v8", tag="v8")
            s_iter(wl8_sb[:], 0.1 / SW, v8)
            mc8 = wp.tile([128, NCH, NO], f8, name="mc8", tag="mc8")
            b_update(v8, first=True, mc_half=mc_half_fn(mc8, wl84))

            # ---------------- iteration 2 ----------------
            v8 = wp.tile([128, 2, NO], f8, name="v8b", tag="v8")
            s_iter(mc8[:], 1.0 / SW, v8)
            mc3 = wp.tile([128, NCH, NO], bf16, name="mc3", tag="mc3")
            b_update(v8, first=False, mc_half=mc_half_fn(mc3, wlb4))

            # ---------------- iteration 3: bf16, own batch shard ----------
            s3_ps = ps_s.tile([B_SH, NO], f32, name="s3_ps", tag="s3_ps")
            for c in range(NCH):
                nc.tensor.matmul(s3_ps[:], xts_sb[:, c, :], mc3[:, c, :],
                                 start=(c == 0), stop=(c == NCH - 1))
            ssh = wp.tile([B_SH, 1, NO], f32, name="ssh", tag="ssh")
            nc.scalar.copy(ssh[:, 0, :], s3_ps[:])
            ysh = wp.tile([B_SH, 1, NO], f32, name="ysh", tag="ysh")
            squash_half(ssh[:], ysh[:], B_SH, 1, "s", 1.0, newton_iters=2)
            nc.sync.dma_start(y_d[:], ysh[:, 0, :])

    nc.compile()
    return nc


def _pack_pm(arr2d, cols):
    """[J, cols] row-major -> [128, NCH*cols] partition-major contiguous."""
    return np.ascontiguousarray(
        arr2d.reshape(NCH, 128, cols).transpose(1, 0, 2).reshape(
            128, NCH * cols))


def _host_prep(x, W):
    """Per-core input dicts; only xts (the 32-col batch shard of x, bf16)
    differs between cores."""
    import ml_dtypes

    bf = ml_dtypes.bfloat16
    f8 = ml_dtypes.float8_e4m3
    x = np.ascontiguousarray(x, dtype=np.float32)
    W = np.ascontiguousarray(W, dtype=np.float32)
    xt = np.ascontiguousarray(x.transpose(2, 1, 0)).reshape(J, B)
    xik = np.ascontiguousarray(x.transpose(0, 2, 1)).reshape(B, J)
    wl = np.ascontiguousarray(
        (np.float32(0.03) * W[0]).transpose(0, 3, 1, 2)).reshape(J, NO)
    xt8 = _pack_pm(xt.astype(f8), B)
    xik8 = np.ascontiguousarray(
        xik.astype(f8).reshape(2, 128, J).transpose(1, 0, 2).reshape(
            128, 2 * J))
    wl8 = _pack_pm((wl * np.float32(SW)).astype(f8), NO)
    wlb = _pack_pm(wl.astype(bf), NO)
    xtb = xt.astype(bf)
    # F entries 1/(B*SV) = 2^-12: exact in bf16.
    F = (np.kron(np.eye(16, dtype=np.float32),
                 np.ones((8, 8), dtype=np.float32))
         / np.float32(B * SV)).astype(bf)
    base = {"xt8": xt8, "xik8": xik8, "wl8": wl8, "wlb": wlb, "fmat": F}
    in_maps = []
    for c in range(N_CORES):
        m = dict(base)
        m["xts"] = _pack_pm(np.ascontiguousarray(
            xtb[:, c * B_SH:(c + 1) * B_SH]), B_SH)
        in_maps.append(m)
    return in_maps


def _run(in_maps, trace=False, all_cores=False):
    from concourse.bass_utils import run_bass_kernel_spmd

    if "nc" not in _CACHE:
        _CACHE["nc"] = _build_program()
    nc = _CACHE["nc"]
    kwargs = {}
    if all_cores:
        kwargs["trace_cores"] = list(range(N_CORES))
    res = run_bass_kernel_spmd(nc, in_maps, core_ids=list(range(N_CORES)),
                               trace=trace, **kwargs)
    return res


def kernel(x: np.ndarray, W: np.ndarray) -> np.ndarray:
    in_maps = _host_prep(x, W)
    res = _run(in_maps)
    v = np.concatenate([res.results[c]["y"] for c in range(N_CORES)], axis=0)
    return v.reshape(B, N_NODE, O_SZ, 1).astype(np.float32)
